# revision 9
# baseline (speedup 1.0000x reference)
"""DiffLogic 3-layer network on 8 Trainium2 NeuronCores.

Strategy (data-parallel over batch, per spec hint):
  - Each core gets 512 of the 4096 batch rows. Weights/indices replicated.
  - Activations kept feature-major ([features, batch] fp16) so the
    per-layer feature gathers become row gathers served by gpsimd dma_gather
    (random rows from DRAM -> SBUF partitions).
  - Layer output rows are permuted (host-side index rewiring) so each
    layer's a-operand gather reads DRAM rows in ascending order (HBM
    locality); intermediate h tensors are stored partition-major.
  - Soft-logic coefficients c_k = softmax(w) @ OP_COEF are precomputed on
    the host and shipped as inputs (weight-only preprocessing), removing
    the on-device softmax preamble.
  - The soft-logic mixture out = (c3*b + c1)*a + (c2*b + c0) runs on
    DVE (tensor_scalar / tensor_tensor) + ACT (affine) with per-partition
    coefficient scalars.
  - L3 optimizations vs the 4-matmul baseline:
      * h2 is stored in fp8 (e4m3) via cast-on-DMA; L3's gathers move half
        the bytes. The only elementwise op on fp8 is abg = a*b (DVE).
      * The group-sum's linear terms sum_o c1[o]*h2[ia3[o]] + c2[o]*h2[ib3[o]]
        collapse to a dense [10, 8192] matmul W_ab @ h2, executed against
        the fp16 h2 tiles during L2's epilogue (no gather needed).
      * The c3 (a*b) term uses one fp8 matmul per chunk with c3-scaled
        one-hot stationaries.
      * The c0 term folds into the final activation's per-partition bias.
"""

import numpy as np

# ---- problem constants (hardcoded per contract) ----
B, D0, D1, D2, D3 = 4096, 1024, 8192, 8192, 10240
NCORES = 8
BS = B // NCORES  # 512 batch rows per core
K = 10
TAU = 30.0

LAYERS = [
    # (n_out, n_src)
    (D1, D0),
    (D2, D1),
    (D3, D2),
]
NCH = [o // 128 for o, _ in LAYERS]  # [64, 64, 80]
NCH12 = NCH[0] + NCH[1]  # L1+L2 chunks carrying ck coefficients
CH_OFF = [0, NCH[0]]  # ck chunk offsets for L1/L2
GRP = 8  # chunks per dma_gather (8*128 = 1024 rows per gather)
ACT_T1_MOD = 6  # chunks with lc % 8 < this run the t1 affine on ACT

_OP_COEF = np.array([
    [0., 0., 0., 0.], [0., 0., 0., 1.], [0., 1., 0., -1.], [0., 1., 0., 0.],
    [0., 0., 1., -1.], [0., 0., 1., 0.], [0., 1., 1., -2.], [0., 1., 1., -1.],
    [1., -1., -1., 1.], [1., -1., -1., 2.], [1., 0., -1., 0.], [1., 0., -1., 1.],
    [1., -1., 0., 0.], [1., -1., 0., 1.], [1., 0., 0., -1.], [1., 0., 0., 0.],
], dtype=np.float32)

_nc_cache = {}


def _build_nc(repeat=1, act_t1_mod=ACT_T1_MOD, tt_group=4, grp=GRP, nq=3, l3_fp8=1, gbufs=3):
    from concourse import bacc, bass, mybir
    from concourse.tile import TileContext

    f16 = mybir.dt.float16
    f32 = mybir.dt.float32
    f8 = mybir.dt.float8e4
    i16 = mybir.dt.int16
    Alu = mybir.AluOpType
    Act = mybir.ActivationFunctionType

    nc = bacc.Bacc(None, target_bir_lowering=False, num_swdge_queues=nq)

    # ---- I/O ----
    xT = nc.dram_tensor("xT", [D0, BS], f16, kind="ExternalInput")
    ck_in = nc.dram_tensor("ckall", [128, 4, NCH12], f32, kind="ExternalInput")
    idx_in = []
    for li, (o, _) in enumerate(LAYERS):
        ia = nc.dram_tensor(f"ia{li}", [128, o // 16], i16, kind="ExternalInput")
        ib = nc.dram_tensor(f"ib{li}", [128, o // 16], i16, kind="ExternalInput")
        idx_in.append((ia, ib))
    f8l3 = f8 if l3_fp8 else f16
    wab_in = nc.dram_tensor("wabT", [128, NCH[1] * K], f16, kind="ExternalInput")
    selk3_in = nc.dram_tensor("selk3", [128, NCH[2] * K], f8l3, kind="ExternalInput")
    bias0_in = nc.dram_tensor("bias0", [K, 1], f32, kind="ExternalInput")
    out_d = nc.dram_tensor("out", [K, BS], f32, kind="ExternalOutput")

    # intermediate activations, partition-major: h[p, c, b] = row (p*c_n + c)
    h1_d = nc.dram_tensor("h1", [128, NCH[0], BS], f16)
    h2_d = nc.dram_tensor("h2", [128, NCH[1], BS], f8l3)
    src_ap = [
        lambda: xT[:],
        lambda: h1_d[:].rearrange("p c b -> (p c) b"),
        lambda: h2_d[:].rearrange("p c b -> (p c) b"),
    ]

    with TileContext(nc) as tc:
      for _rep in range(repeat):
        with (
            tc.tile_pool(name="pers", bufs=1) as pers,
            tc.tile_pool(name="psum", bufs=1, space="PSUM") as psump,
        ):
            ck = pers.tile([128, 4, NCH12], f32, name="ck")
            nc.sync.dma_start(out=ck[:], in_=ck_in[:])
            wab_t = pers.tile([128, NCH[1] * K], f16, name="wab")
            nc.sync.dma_start(out=wab_t[:], in_=wab_in[:])
            selk3_t = pers.tile([128, NCH[2] * K], f8l3, name="selk3")
            nc.sync.dma_start(out=selk3_t[:], in_=selk3_in[:])
            bias0_t = pers.tile([K, 1], f32, name="bias0")
            nc.sync.dma_start(out=bias0_t[:], in_=bias0_in[:])

            psum_out = psump.tile([K, BS], f32, space="PSUM")
            with (
                tc.tile_pool(name="idxp", bufs=2) as idxp,
                tc.tile_pool(name="gath", bufs=gbufs) as gath,
                tc.tile_pool(name="outp", bufs=3) as outp,
                tc.tile_pool(name="tmp", bufs=3) as tmp,
            ):
                for li, (o, n_src) in enumerate(LAYERS):
                    nch = NCH[li]
                    ia_t = idxp.tile([128, o // 16], i16, tag="ia")
                    nc.sync.dma_start(out=ia_t[:], in_=idx_in[li][0][:])
                    ib_t = idxp.tile([128, o // 16], i16, tag="ib")
                    nc.sync.dma_start(out=ib_t[:], in_=idx_in[li][1][:])

                    ldt = f8l3 if li == 2 else f16
                    for g in range(nch // grp):
                        gA = gath.tile([128, grp, BS], ldt, tag="gA")
                        nc.gpsimd.dma_gather(
                            out_ap=gA[:],
                            in_ap=src_ap[li](),
                            idxs_ap=ia_t[:, g * grp * 8 : (g + 1) * grp * 8],
                            num_idxs=grp * 128,
                            num_idxs_reg=grp * 128,
                            elem_size=BS,
                            single_packet=False,
                            queue_num=1 + (2 * g) % (nq - 1),
                        )
                        gB = gath.tile([128, grp, BS], ldt, tag="gB")
                        nc.gpsimd.dma_gather(
                            out_ap=gB[:],
                            in_ap=src_ap[li](),
                            idxs_ap=ib_t[:, g * grp * 8 : (g + 1) * grp * 8],
                            num_idxs=grp * 128,
                            num_idxs_reg=grp * 128,
                            elem_size=BS,
                            single_packet=False,
                            queue_num=1 + (2 * g + 1) % (nq - 1),
                        )
                        if li == 2:
                            # c3 * (a*b) term: elementwise product (fp8) then
                            # one matmul per chunk with c3-scaled one-hots.
                            abg = gath.tile([128, grp, BS], f8l3, tag="abg")
                            nc.vector.tensor_tensor(
                                out=abg[:], in0=gA[:], in1=gB[:], op=Alu.mult
                            )
                            for c in range(grp):
                                lc = g * grp + c
                                nc.tensor.matmul(
                                    out=psum_out[:],
                                    lhsT=selk3_t[:, lc * K : (lc + 1) * K],
                                    rhs=abg[:, c, :],
                                    start=False,
                                    stop=(lc == NCH[2] - 1),
                                )
                            continue
                        ho = outp.tile([128, grp, BS], f16, tag="ho")
                        TG = tt_group
                        for cg in range(grp // TG):
                            t1g = tmp.tile([128, TG, BS], f16, tag="t1")
                            t3g = tmp.tile([128, TG, BS], f16, tag="t3")
                            for c4 in range(TG):
                                c = cg * TG + c4
                                lc = g * grp + c  # layer-local chunk
                                gc = CH_OFF[li] + lc  # ck chunk
                                b = gB[:, c, :]
                                # t1 = c3*b + c1
                                if lc % 8 < act_t1_mod:
                                    nc.scalar.activation(
                                        out=t1g[:, c4, :],
                                        in_=b,
                                        func=Act.Identity,
                                        scale=ck[:, 3, gc : gc + 1],
                                        bias=ck[:, 1, gc : gc + 1],
                                    )
                                else:
                                    nc.vector.tensor_scalar(
                                        out=t1g[:, c4, :],
                                        in0=b,
                                        scalar1=ck[:, 3, gc : gc + 1],
                                        scalar2=ck[:, 1, gc : gc + 1],
                                        op0=Alu.mult,
                                        op1=Alu.add,
                                    )
                                # t3 = c2*b + c0   (ACT affine)
                                nc.scalar.activation(
                                    out=t3g[:, c4, :],
                                    in_=b,
                                    func=Act.Identity,
                                    scale=ck[:, 2, gc : gc + 1],
                                    bias=ck[:, 0, gc : gc + 1],
                                )
                            # t2 = t1 * a ; ho = t2 + t3 (grouped over TG chunks)
                            t2g = tmp.tile([128, TG, BS], f16, tag="t2")
                            nc.vector.tensor_tensor(
                                out=t2g[:],
                                in0=t1g[:],
                                in1=gA[:, cg * TG : (cg + 1) * TG, :],
                                op=Alu.mult,
                            )
                            nc.vector.tensor_tensor(
                                out=ho[:, cg * TG : (cg + 1) * TG, :],
                                in0=t2g[:],
                                in1=t3g[:],
                                op=Alu.add,
                            )
                        if li == 0:
                            nc.sync.dma_start(
                                out=h1_d[:, g * grp : (g + 1) * grp, :], in_=ho[:]
                            )
                        else:
                            # L2 epilogue: linear-term fold matmuls (fp16) and
                            # fp8 cast-on-DMA store for L3's gathers.
                            for c in range(grp):
                                lc = g * grp + c
                                nc.tensor.matmul(
                                    out=psum_out[:],
                                    lhsT=wab_t[:, lc * K : (lc + 1) * K],
                                    rhs=ho[:, c, :],
                                    start=(lc == 0),
                                    stop=False,
                                )
                            if l3_fp8:
                                nc.gpsimd.dma_start(
                                    out=h2_d[:, g * grp : (g + 1) * grp, :],
                                    in_=ho[:],
                                )
                            else:
                                nc.sync.dma_start(
                                    out=h2_d[:, g * grp : (g + 1) * grp, :],
                                    in_=ho[:],
                                )

            out_sb = pers.tile([K, BS], f32)
            nc.scalar.activation(
                out=out_sb[:],
                in_=psum_out[:],
                func=Act.Identity,
                scale=1.0 / TAU,
                bias=bias0_t[:],
            )
            nc.sync.dma_start(out=out_d[:], in_=out_sb[:])

    nc.compile()
    return nc


def _wrap_idx(idx: np.ndarray) -> np.ndarray:
    """int16 index layout for dma_gather: wrapped in 16 partitions,
    replicated to 128 partitions (8 gpsimd cores)."""
    n = idx.shape[0]
    blk = idx.astype(np.int16).reshape(n // 16, 16).T  # [16, n/16]
    return np.ascontiguousarray(np.tile(blk, (8, 1)))  # [128, n/16]


def _coefs(w: np.ndarray) -> np.ndarray:
    w = w.astype(np.float32)
    e = np.exp(w - w.max(-1, keepdims=True))
    p = e / e.sum(-1, keepdims=True)
    return p @ _OP_COEF  # [O, 4]


def _prep_shared(w1, w2, w3, idx_a1, idx_b1, idx_a2, idx_b2, idx_a3, idx_b3,
                 l3_fp8=1):
    """Host-side prep: softmax coefficients, per-layer output-row permutation
    sigma (sorting the a-gather), source-row remap pi into the stored layout,
    L3 linear-term fold matrix, c3 one-hot stationaries, c0 bias."""
    from ml_dtypes import float8_e4m3

    ws = (w1, w2, w3)
    ias = (idx_a1, idx_a2, idx_a3)
    ibs = (idx_b1, idx_b2, idx_b3)

    shared = {}
    ck_parts = []
    pi_prev = None  # original source row -> stored virtual row
    coef3_s = ia3_s = ib3_s = None
    for li in range(3):
        o = LAYERS[li][0]
        nch = NCH[li]
        ia = ias[li].astype(np.int64)
        ib = ibs[li].astype(np.int64)
        if pi_prev is not None:
            ia = pi_prev[ia]
            ib = pi_prev[ib]
        if li < 2:
            sigma = np.argsort(ia, kind="stable")
        else:
            # keep group structure: sort within each block of 1024 rows
            sigma = np.concatenate(
                [g * 1024 + np.argsort(ia[g * 1024 : (g + 1) * 1024], kind="stable")
                 for g in range(K)]
            )
        ia_s = ia[sigma]
        ib_s = ib[sigma]
        coef_s = _coefs(ws[li][sigma])  # [o, 4] sorted
        shared[f"ia{li}"] = _wrap_idx(ia_s)
        shared[f"ib{li}"] = _wrap_idx(ib_s)
        if li < 2:
            # ck layout [128, 4, nch]: coefficient k of sorted row c*128+p
            ck_parts.append(
                np.ascontiguousarray(
                    coef_s.reshape(nch, 128, 4).transpose(1, 2, 0)
                )
            )
            inv = np.empty(o, np.int64)
            inv[sigma] = np.arange(o)
            pi_prev = (inv % 128) * nch + inv // 128
        else:
            coef3_s, ia3_s, ib3_s = coef_s, ia_s, ib_s

    shared["ckall"] = np.ascontiguousarray(np.concatenate(ck_parts, axis=2))

    # ---- L3 host prep ----
    nch3 = NCH[2]
    grp_of = np.arange(D3) // (D3 // K)  # sorted position -> group (block-sorted)

    # W_ab[k, r] over h2 stored rows r (= p*NCH[1] + c)
    W_ab = np.zeros((K, D2), np.float32)
    np.add.at(W_ab, (grp_of, ia3_s), coef3_s[:, 1])
    np.add.at(W_ab, (grp_of, ib3_s), coef3_s[:, 2])
    # wabT[p, c*K + k] = W_ab[k, p*NCH[1] + c]
    wabT = W_ab.reshape(K, 128, NCH[1]).transpose(1, 2, 0).reshape(128, NCH[1] * K)
    shared["wabT"] = np.ascontiguousarray(wabT.astype(np.float16))

    # selk3[p, lc*K + g(lc)] = c3 of sorted row lc*128+p, fp8
    selk3 = np.zeros((128, nch3 * K), np.float32)
    lc_idx = np.arange(D3) // 128
    p_idx = np.arange(D3) % 128
    selk3[p_idx, lc_idx * K + lc_idx // 8] = coef3_s[:, 3]
    shared["selk3"] = selk3.astype(float8_e4m3 if l3_fp8 else np.float16)

    c0sum = np.zeros(K, np.float32)
    np.add.at(c0sum, grp_of, coef3_s[:, 0])
    shared["bias0"] = (c0sum / TAU).reshape(K, 1).astype(np.float32)
    return shared


def make_in_maps(x, l3_fp8=1, **shared_inputs):
    shared = _prep_shared(l3_fp8=l3_fp8, **shared_inputs)
    in_maps = []
    for c in range(NCORES):
        xs = x[c * BS : (c + 1) * BS].astype(np.float16)  # [512, 1024]
        xT = np.ascontiguousarray(xs.T)  # [1024, 512]
        in_maps.append({"xT": xT, **shared})
    return in_maps


def get_nc(repeat=1, **opts):
    key = (repeat, tuple(sorted(opts.items())))
    if key not in _nc_cache:
        _nc_cache[key] = _build_nc(repeat, **opts)
    return _nc_cache[key]


def kernel(
    x, w1, w2, w3, idx_a1, idx_b1, idx_a2, idx_b2, idx_a3, idx_b3
) -> np.ndarray:
    from concourse.bass_utils import run_bass_kernel_spmd

    nc = get_nc()
    in_maps = make_in_maps(
        np.asarray(x),
        w1=np.asarray(w1),
        w2=np.asarray(w2),
        w3=np.asarray(w3),
        idx_a1=np.asarray(idx_a1),
        idx_b1=np.asarray(idx_b1),
        idx_a2=np.asarray(idx_a2),
        idx_b2=np.asarray(idx_b2),
        idx_a3=np.asarray(idx_a3),
        idx_b3=np.asarray(idx_b3),
    )
    res = run_bass_kernel_spmd(nc, in_maps, core_ids=list(range(NCORES)))
    out = np.empty((B, K), np.float32)
    for c in range(NCORES):
        out[c * BS : (c + 1) * BS] = res.results[c]["out"].T
    return out


# revision 10
# speedup vs baseline: 1.3649x; 1.3649x over previous
"""DiffLogic 3-layer network on 8 Trainium2 NeuronCores.

Strategy (data-parallel over batch, per spec hint):
  - Each core gets 512 of the 4096 batch rows. Weights/indices replicated.
  - Activations kept feature-major ([features, batch] fp16) so the
    per-layer feature gathers become row gathers served by gpsimd dma_gather
    (random rows from DRAM -> SBUF partitions).
  - Layer output rows are permuted (host-side index rewiring) so each
    layer's a-operand gather reads DRAM rows in ascending order (HBM
    locality); intermediate h tensors are stored partition-major.
  - Soft-logic coefficients c_k = softmax(w) @ OP_COEF are precomputed on
    the host and shipped as inputs (weight-only preprocessing), removing
    the on-device softmax preamble.
  - The soft-logic mixture out = (c3*b + c1)*a + (c2*b + c0) runs on
    DVE (tensor_scalar / tensor_tensor) + ACT (affine) with per-partition
    coefficient scalars.
  - L3 optimizations vs the 4-matmul baseline:
      * h2 is stored in fp8 (e4m3) via cast-on-DMA; L3's gathers move half
        the bytes. The only elementwise op on fp8 is abg = a*b (DVE).
      * The group-sum's linear terms sum_o c1[o]*h2[ia3[o]] + c2[o]*h2[ib3[o]]
        collapse to a dense [10, 8192] matmul W_ab @ h2, executed against
        the fp16 h2 tiles during L2's epilogue (no gather needed).
      * The c3 (a*b) term uses one fp8 matmul per chunk with c3-scaled
        one-hot stationaries.
      * The c0 term folds into the final activation's per-partition bias.
"""

import numpy as np

# ---- problem constants (hardcoded per contract) ----
B, D0, D1, D2, D3 = 4096, 1024, 8192, 8192, 10240
NCORES = 8
BS = B // NCORES  # 512 batch rows per core
K = 10
TAU = 30.0

LAYERS = [
    # (n_out, n_src)
    (D1, D0),
    (D2, D1),
    (D3, D2),
]
NCH = [o // 128 for o, _ in LAYERS]  # [64, 64, 80]
NCH12 = NCH[0] + NCH[1]  # L1+L2 chunks carrying ck coefficients
CH_OFF = [0, NCH[0]]  # ck chunk offsets for L1/L2
GRP = 8  # chunks per dma_gather (8*128 = 1024 rows per gather)
ACT_T1_MOD = 6  # chunks with lc % 8 < this run the t1 affine on ACT

_OP_COEF = np.array([
    [0., 0., 0., 0.], [0., 0., 0., 1.], [0., 1., 0., -1.], [0., 1., 0., 0.],
    [0., 0., 1., -1.], [0., 0., 1., 0.], [0., 1., 1., -2.], [0., 1., 1., -1.],
    [1., -1., -1., 1.], [1., -1., -1., 2.], [1., 0., -1., 0.], [1., 0., -1., 1.],
    [1., -1., 0., 0.], [1., -1., 0., 1.], [1., 0., 0., -1.], [1., 0., 0., 0.],
], dtype=np.float32)

_nc_cache = {}


def _build_nc(repeat=1, act_t1_mod=ACT_T1_MOD, tt_group=4, grp=GRP, nq=3, l3_fp8=1, gbufs=3, probe=0):
    from concourse import bacc, bass, mybir
    from concourse.tile import TileContext

    f16 = mybir.dt.float16
    f32 = mybir.dt.float32
    f8 = mybir.dt.float8e4
    i16 = mybir.dt.int16
    Alu = mybir.AluOpType
    Act = mybir.ActivationFunctionType

    nc = bacc.Bacc(None, target_bir_lowering=False, num_swdge_queues=nq)

    # ---- I/O ----
    xT = nc.dram_tensor("xT", [D0, BS], f16, kind="ExternalInput")
    ck_in = nc.dram_tensor("ckall", [128, 4, NCH12], f32, kind="ExternalInput")
    idx_in = []
    for li, (o, _) in enumerate(LAYERS):
        ia = nc.dram_tensor(f"ia{li}", [128, o // 16], i16, kind="ExternalInput")
        ib = nc.dram_tensor(f"ib{li}", [128, o // 16], i16, kind="ExternalInput")
        idx_in.append((ia, ib))
    f8l3 = f8 if l3_fp8 else f16
    wab_in = nc.dram_tensor("wabT", [128, NCH[1] * K], f16, kind="ExternalInput")
    selk3_in = nc.dram_tensor("selk3", [128, NCH[2] * K], f8l3, kind="ExternalInput")
    bias0_in = nc.dram_tensor("bias0", [K, 1], f32, kind="ExternalInput")
    out_d = nc.dram_tensor("out", [K, BS], f32, kind="ExternalOutput")

    # intermediate activations, partition-major: h[p, c, b] = row (p*c_n + c)
    h1_d = nc.dram_tensor("h1", [128, NCH[0], BS], f16)
    h2_d = nc.dram_tensor("h2", [128, NCH[1], BS], f8l3)
    src_ap = [
        lambda: xT[:],
        lambda: h1_d[:].rearrange("p c b -> (p c) b"),
        lambda: h2_d[:].rearrange("p c b -> (p c) b"),
    ]

    with TileContext(nc) as tc:
      for _rep in range(repeat):
        with (
            tc.tile_pool(name="pers", bufs=1) as pers,
            tc.tile_pool(name="psum", bufs=1, space="PSUM") as psump,
        ):
            ck = pers.tile([128, 4, NCH12], f32, name="ck")
            nc.sync.dma_start(out=ck[:], in_=ck_in[:])
            wab_t = pers.tile([128, NCH[1] * K], f16, name="wab")
            nc.sync.dma_start(out=wab_t[:], in_=wab_in[:])
            selk3_t = pers.tile([128, NCH[2] * K], f8l3, name="selk3")
            nc.sync.dma_start(out=selk3_t[:], in_=selk3_in[:])
            bias0_t = pers.tile([K, 1], f32, name="bias0")
            nc.sync.dma_start(out=bias0_t[:], in_=bias0_in[:])

            psum_out = psump.tile([K, BS], f32, space="PSUM")
            with (
                tc.tile_pool(name="idxp", bufs=2) as idxp,
                tc.tile_pool(name="gath", bufs=gbufs) as gath,
                tc.tile_pool(name="outp", bufs=3) as outp,
                tc.tile_pool(name="tmp", bufs=3) as tmp,
            ):
                for li, (o, n_src) in enumerate(LAYERS):
                    nch = NCH[li]
                    ia_t = idxp.tile([128, o // 16], i16, tag="ia")
                    nc.sync.dma_start(out=ia_t[:], in_=idx_in[li][0][:])
                    ib_t = idxp.tile([128, o // 16], i16, tag="ib")
                    nc.sync.dma_start(out=ib_t[:], in_=idx_in[li][1][:])

                    ldt = f8l3 if li == 2 else f16
                    for g in range(nch // grp):
                        gA = gath.tile([128, grp, BS], ldt, tag="gA")
                        nc.gpsimd.dma_gather(
                            out_ap=gA[:],
                            in_ap=src_ap[li](),
                            idxs_ap=ia_t[:, g * grp * 8 : (g + 1) * grp * 8],
                            num_idxs=grp * 128,
                            num_idxs_reg=grp * 128,
                            elem_size=BS,
                            single_packet=False,
                            queue_num=1 + (2 * g) % (nq - 1),
                        )
                        gB = gath.tile([128, grp, BS], ldt, tag="gB")
                        nc.gpsimd.dma_gather(
                            out_ap=gB[:],
                            in_ap=src_ap[li](),
                            idxs_ap=(ia_t if probe == 1 else ib_t)[:, g * grp * 8 : (g + 1) * grp * 8],
                            num_idxs=grp * 128,
                            num_idxs_reg=grp * 128,
                            elem_size=BS,
                            single_packet=False,
                            queue_num=1 + (2 * g + 1) % (nq - 1),
                        )
                        if li == 2:
                            # c3 * (a*b) term: elementwise product (fp8) then
                            # one matmul per chunk with c3-scaled one-hots.
                            abg = gath.tile([128, grp, BS], f8l3, tag="abg")
                            nc.vector.tensor_tensor(
                                out=abg[:], in0=gA[:], in1=gB[:], op=Alu.mult
                            )
                            for c in range(grp):
                                lc = g * grp + c
                                nc.tensor.matmul(
                                    out=psum_out[:],
                                    lhsT=selk3_t[:, lc * K : (lc + 1) * K],
                                    rhs=abg[:, c, :],
                                    start=False,
                                    stop=(lc == NCH[2] - 1),
                                )
                            continue
                        ho = outp.tile([128, grp, BS], f16, tag="ho")
                        TG = tt_group
                        for cg in range(grp // TG):
                            t1g = tmp.tile([128, TG, BS], f16, tag="t1")
                            t3g = tmp.tile([128, TG, BS], f16, tag="t3")
                            for c4 in range(TG):
                                c = cg * TG + c4
                                lc = g * grp + c  # layer-local chunk
                                gc = CH_OFF[li] + lc  # ck chunk
                                b = gB[:, c, :]
                                # t1 = c3*b + c1
                                if lc % 8 < act_t1_mod:
                                    nc.scalar.activation(
                                        out=t1g[:, c4, :],
                                        in_=b,
                                        func=Act.Identity,
                                        scale=ck[:, 3, gc : gc + 1],
                                        bias=ck[:, 1, gc : gc + 1],
                                    )
                                else:
                                    nc.vector.tensor_scalar(
                                        out=t1g[:, c4, :],
                                        in0=b,
                                        scalar1=ck[:, 3, gc : gc + 1],
                                        scalar2=ck[:, 1, gc : gc + 1],
                                        op0=Alu.mult,
                                        op1=Alu.add,
                                    )
                                # t3 = c2*b + c0   (ACT affine)
                                nc.scalar.activation(
                                    out=t3g[:, c4, :],
                                    in_=b,
                                    func=Act.Identity,
                                    scale=ck[:, 2, gc : gc + 1],
                                    bias=ck[:, 0, gc : gc + 1],
                                )
                            # t2 = t1 * a ; ho = t2 + t3 (grouped over TG chunks)
                            t2g = tmp.tile([128, TG, BS], f16, tag="t2")
                            nc.vector.tensor_tensor(
                                out=t2g[:],
                                in0=t1g[:],
                                in1=gA[:, cg * TG : (cg + 1) * TG, :],
                                op=Alu.mult,
                            )
                            nc.vector.tensor_tensor(
                                out=ho[:, cg * TG : (cg + 1) * TG, :],
                                in0=t2g[:],
                                in1=t3g[:],
                                op=Alu.add,
                            )
                        if li == 0:
                            nc.sync.dma_start(
                                out=h1_d[:, g * grp : (g + 1) * grp, :], in_=ho[:]
                            )
                        else:
                            # L2 epilogue: linear-term fold matmuls (fp16) and
                            # fp8 cast-on-DMA store for L3's gathers.
                            for c in range(grp):
                                lc = g * grp + c
                                nc.tensor.matmul(
                                    out=psum_out[:],
                                    lhsT=wab_t[:, lc * K : (lc + 1) * K],
                                    rhs=ho[:, c, :],
                                    start=(lc == 0),
                                    stop=False,
                                )
                            if l3_fp8:
                                nc.gpsimd.dma_start(
                                    out=h2_d[:, g * grp : (g + 1) * grp, :],
                                    in_=ho[:],
                                )
                            else:
                                nc.sync.dma_start(
                                    out=h2_d[:, g * grp : (g + 1) * grp, :],
                                    in_=ho[:],
                                )

            out_sb = pers.tile([K, BS], f32)
            nc.scalar.activation(
                out=out_sb[:],
                in_=psum_out[:],
                func=Act.Identity,
                scale=1.0 / TAU,
                bias=bias0_t[:],
            )
            nc.sync.dma_start(out=out_d[:], in_=out_sb[:])

    nc.compile()
    return nc


def _wrap_idx(idx: np.ndarray) -> np.ndarray:
    """int16 index layout for dma_gather: wrapped in 16 partitions,
    replicated to 128 partitions (8 gpsimd cores)."""
    n = idx.shape[0]
    blk = idx.astype(np.int16).reshape(n // 16, 16).T  # [16, n/16]
    return np.ascontiguousarray(np.tile(blk, (8, 1)))  # [128, n/16]


def _coefs(w: np.ndarray) -> np.ndarray:
    w = w.astype(np.float32)
    e = np.exp(w - w.max(-1, keepdims=True))
    p = e / e.sum(-1, keepdims=True)
    return p @ _OP_COEF  # [O, 4]


def _prep_shared(w1, w2, w3, idx_a1, idx_b1, idx_a2, idx_b2, idx_a3, idx_b3,
                 l3_fp8=1):
    """Host-side prep: softmax coefficients, per-layer output-row permutation
    sigma (sorting the a-gather), source-row remap pi into the stored layout,
    L3 linear-term fold matrix, c3 one-hot stationaries, c0 bias."""
    from ml_dtypes import float8_e4m3

    ws = (w1, w2, w3)
    ias = (idx_a1, idx_a2, idx_a3)
    ibs = (idx_b1, idx_b2, idx_b3)

    shared = {}
    ck_parts = []
    pi_prev = None  # original source row -> stored virtual row
    coef3_s = ia3_s = ib3_s = None
    for li in range(3):
        o = LAYERS[li][0]
        nch = NCH[li]
        ia = ias[li].astype(np.int64)
        ib = ibs[li].astype(np.int64)
        if pi_prev is not None:
            ia = pi_prev[ia]
            ib = pi_prev[ib]
        if li < 2:
            sigma = np.argsort(ia, kind="stable")
        else:
            # keep group structure: sort within each block of 1024 rows
            sigma = np.concatenate(
                [g * 1024 + np.argsort(ia[g * 1024 : (g + 1) * 1024], kind="stable")
                 for g in range(K)]
            )
        ia_s = ia[sigma]
        ib_s = ib[sigma]
        coef_s = _coefs(ws[li][sigma])  # [o, 4] sorted
        shared[f"ia{li}"] = _wrap_idx(ia_s)
        shared[f"ib{li}"] = _wrap_idx(ib_s)
        if li < 2:
            # ck layout [128, 4, nch]: coefficient k of sorted row c*128+p
            ck_parts.append(
                np.ascontiguousarray(
                    coef_s.reshape(nch, 128, 4).transpose(1, 2, 0)
                )
            )
            inv = np.empty(o, np.int64)
            inv[sigma] = np.arange(o)
            pi_prev = (inv % 128) * nch + inv // 128
        else:
            coef3_s, ia3_s, ib3_s = coef_s, ia_s, ib_s

    shared["ckall"] = np.ascontiguousarray(np.concatenate(ck_parts, axis=2))

    # ---- L3 host prep ----
    nch3 = NCH[2]
    grp_of = np.arange(D3) // (D3 // K)  # sorted position -> group (block-sorted)

    # W_ab[k, r] over h2 stored rows r (= p*NCH[1] + c)
    W_ab = np.zeros((K, D2), np.float32)
    np.add.at(W_ab, (grp_of, ia3_s), coef3_s[:, 1])
    np.add.at(W_ab, (grp_of, ib3_s), coef3_s[:, 2])
    # wabT[p, c*K + k] = W_ab[k, p*NCH[1] + c]
    wabT = W_ab.reshape(K, 128, NCH[1]).transpose(1, 2, 0).reshape(128, NCH[1] * K)
    shared["wabT"] = np.ascontiguousarray(wabT.astype(np.float16))

    # selk3[p, lc*K + g(lc)] = c3 of sorted row lc*128+p, fp8
    selk3 = np.zeros((128, nch3 * K), np.float32)
    lc_idx = np.arange(D3) // 128
    p_idx = np.arange(D3) % 128
    selk3[p_idx, lc_idx * K + lc_idx // 8] = coef3_s[:, 3]
    shared["selk3"] = selk3.astype(float8_e4m3 if l3_fp8 else np.float16)

    c0sum = np.zeros(K, np.float32)
    np.add.at(c0sum, grp_of, coef3_s[:, 0])
    shared["bias0"] = (c0sum / TAU).reshape(K, 1).astype(np.float32)
    return shared


def make_in_maps(x, l3_fp8=1, **shared_inputs):
    shared = _prep_shared(l3_fp8=l3_fp8, **shared_inputs)
    in_maps = []
    for c in range(NCORES):
        xs = x[c * BS : (c + 1) * BS].astype(np.float16)  # [512, 1024]
        xT = np.ascontiguousarray(xs.T)  # [1024, 512]
        in_maps.append({"xT": xT, **shared})
    return in_maps


def get_nc(repeat=1, **opts):
    key = (repeat, tuple(sorted(opts.items())))
    if key not in _nc_cache:
        _nc_cache[key] = _build_nc(repeat, **opts)
    return _nc_cache[key]


def kernel(
    x, w1, w2, w3, idx_a1, idx_b1, idx_a2, idx_b2, idx_a3, idx_b3
) -> np.ndarray:
    from concourse.bass_utils import run_bass_kernel_spmd

    nc = get_nc()
    in_maps = make_in_maps(
        np.asarray(x),
        w1=np.asarray(w1),
        w2=np.asarray(w2),
        w3=np.asarray(w3),
        idx_a1=np.asarray(idx_a1),
        idx_b1=np.asarray(idx_b1),
        idx_a2=np.asarray(idx_a2),
        idx_b2=np.asarray(idx_b2),
        idx_a3=np.asarray(idx_a3),
        idx_b3=np.asarray(idx_b3),
    )
    res = run_bass_kernel_spmd(nc, in_maps, core_ids=list(range(NCORES)))
    out = np.empty((B, K), np.float32)
    for c in range(NCORES):
        out[c * BS : (c + 1) * BS] = res.results[c]["out"].T
    return out
